# revision 8
# baseline (speedup 1.0000x reference)
"""Trainium2 Bass kernel for nn_AttentionLayer (GAT-style masked attention).

Computes, for full inputs:
    h1 = emb_src @ W                      [8000, 128]
    g  = emb_dest @ (W @ W2)              [10000, 128]
    e  = g @ h1.T                         [10000, 8000]
    s  = lrelu(e, 0.2) * (1/sqrt(128))    masked to -inf where bias <= 0
    att = softmax(s, axis=1)
    out = att @ ft                        [10000, 1]   (ft = nan-cleaned feature_src)

Sharding: N_dest split across 8 NeuronCores (1250 rows each); emb_src /
feature_src / W / W2 replicated. No collectives. Softmax is computed
unnormalized (numer/denom) — no max-subtraction needed since |s| <= ~10.

Per-core device pipeline (per 128-row dest tile x 1000-col src slice):
    PE:     e_psum = gT.T @ h1T                       (bf16 x bf16 -> f32 PSUM)
    GPSIMD: amask  = (bias <= 0) * -1e30              (from streamed bias tile)
    DVE:    ms     = e_psum + amask
    ACT:    t      = Lrelu(SCALE * ms)  [alpha=0.2]
    ACT:    u      = Exp(t)             [accum_out -> denom partial]
    DVE:    ttr u * ft_bcast            [accum_out -> numer partial]
    out = numer / denom
"""
import os
import sys

sys.path.insert(0, "/opt/trn_rl_repo")

import numpy as np

_CACHE = {}

N_DEST, N_SRC, IN_DIM, HID = 10000, 8000, 256, 128
N_CORES = 8
ND = N_DEST // N_CORES            # 1250 dest rows per core
SCALE = float(1.0 / np.sqrt(np.float32(HID)))

# dest tiles per core: 9 x 128 + 98
DEST_TILES = [(i * 128, min(128, ND - i * 128)) for i in range((ND + 127) // 128)]
SRC_CHUNK = 500                   # matmul N (<= 512 = one PSUM bank of f32)
N_SRC_CHUNKS = N_SRC // SRC_CHUNK # 16
SS_W = 2 * SRC_CHUNK              # 1000-col superslice for ACT/DVE ops
N_SS = N_SRC // SS_W              # 8


def _build_nc():
    import concourse.bass as bass
    import concourse.tile as tile
    from concourse import bacc, mybir
    from concourse.masks import make_identity
    from contextlib import ExitStack

    F32 = mybir.dt.float32
    BF16 = mybir.dt.bfloat16
    FP16 = mybir.dt.float16
    AF = mybir.ActivationFunctionType
    OP = mybir.AluOpType

    nc = bacc.Bacc("TRN2", target_bir_lowering=False, debug=False,
                   num_devices=N_CORES)

    bias_t = nc.declare_dram_parameter("bias", [ND, N_SRC], F32, isOutput=False)
    dest_t = nc.declare_dram_parameter("emb_dest", [ND, IN_DIM], F32, isOutput=False)
    src_t = nc.declare_dram_parameter("emb_src", [N_SRC, IN_DIM], F32, isOutput=False)
    ft_t = nc.declare_dram_parameter("feature_src", [N_SRC, 1], F32, isOutput=False)
    w_t = nc.declare_dram_parameter("W", [IN_DIM, HID], F32, isOutput=False)
    w2_t = nc.declare_dram_parameter("W2", [HID, HID], F32, isOutput=False)
    out_t = nc.declare_dram_parameter("out", [ND, 1], F32, isOutput=True)

    with tile.TileContext(nc) as tc, ExitStack() as ctx:
        persist = ctx.enter_context(tc.tile_pool(name="persist", bufs=1))

        ident = persist.tile([128, 128], F32)
        make_identity(nc, ident)

        # ftbc: feature row broadcast across 128 partitions, then bf16
        ft_row = ft_t[:, :].rearrange("s one -> one s")   # [1, 8000] view
        ftbc = persist.tile([128, N_SRC], F32)
        nc.sync.dma_start(out=ftbc, in_=ft_row.to_broadcast([128, N_SRC]))
        ftbc_bf = persist.tile([128, N_SRC], BF16)
        nc.vector.tensor_copy(out=ftbc_bf, in_=ftbc)

        gt_tiles = []
        h1t_tiles = []

        with tc.tile_pool(name="pre_sb", bufs=3) as pre, \
             tc.tile_pool(name="pre_ps", bufs=2, space="PSUM") as pps:

            # ---- W chunks ([K=in_dim sub, M=hid]) natural + bf16
            w_sb = pre.tile([128, 2, HID], F32, tag="w_sb")
            for c in range(2):
                nc.sync.dma_start(out=w_sb[:, c, :], in_=w_t[128 * c:128 * (c + 1), :])
            w_bf = persist.tile([128, 2, HID], BF16)
            nc.vector.tensor_copy(out=w_bf, in_=w_sb)
            w2_sb = pre.tile([128, HID], F32, tag="w2_sb")
            nc.sync.dma_start(out=w2_sb, in_=w2_t[:, :])

            # ---- Wc = W @ W2, stored as lhsT chunks [K=in_dim sub, M=hid] bf16
            wc_bf = persist.tile([128, 2, HID], BF16)
            for c in range(2):
                ps_tr = pps.tile([128, 128], F32, tag="ps_a")
                nc.tensor.transpose(ps_tr, w_sb[:, c, :], ident)    # [hid, in_sub]
                wTc = pre.tile([128, 128], F32, tag="wTc")
                nc.scalar.copy(out=wTc, in_=ps_tr)
                ps_mm = pps.tile([128, HID], F32, tag="ps_b")
                nc.tensor.matmul(ps_mm, wTc, w2_sb, start=True, stop=True)
                nc.scalar.copy(out=wc_bf[:, c, :], in_=ps_mm)

            # ---- emb_dest -> destT (bf16, [in_sub, 2, dest]) -> gT tiles
            destT = pre.tile([128, 2, ND], BF16, tag="destT")
            for (r0, rn) in DEST_TILES:
                ed = pre.tile([128, IN_DIM], F32, tag="ed")
                nc.sync.dma_start(out=ed[:rn, :], in_=dest_t[r0:r0 + rn, :])
                for c in range(2):
                    ps_tr = pps.tile([128, 128], F32, tag="ps_a")
                    nc.tensor.transpose(ps_tr[:, :rn], ed[:rn, 128 * c:128 * (c + 1)],
                                        ident[:rn, :rn])
                    if c == 0:
                        nc.scalar.copy(out=destT[:, c, r0:r0 + rn], in_=ps_tr[:, :rn])
                    else:
                        nc.vector.tensor_copy(out=destT[:, c, r0:r0 + rn], in_=ps_tr[:, :rn])
            for ti, (r0, rn) in enumerate(DEST_TILES):
                ps_g = pps.tile([128, 128], F32, tag="ps_b")
                for c in range(2):
                    nc.tensor.matmul(ps_g[:, :rn], wc_bf[:, c, :],
                                     destT[:, c, r0:r0 + rn],
                                     start=(c == 0), stop=(c == 1))
                gt = persist.tile([128, 128], BF16, tag=f"gt{ti}")
                nc.scalar.copy(out=gt[:, :rn], in_=ps_g[:, :rn])
                gt_tiles.append(gt)

            # ---- emb_src -> srcT chunks -> h1T chunk tiles
            for j in range(N_SRC_CHUNKS):
                srcT = pre.tile([128, 2, SRC_CHUNK], BF16, tag="srcT")
                for k in range(4):                       # 4 x 125 src rows
                    s0 = j * SRC_CHUNK + k * 125
                    es = pre.tile([125, IN_DIM], F32, tag="es")
                    nc.sync.dma_start(out=es, in_=src_t[s0:s0 + 125, :])
                    for c in range(2):
                        ps_tr = pps.tile([128, 128], F32, tag="ps_a")
                        nc.tensor.transpose(ps_tr[:, :125],
                                            es[:, 128 * c:128 * (c + 1)],
                                            ident[:125, :125])
                        if (k + c) % 2 == 0:
                            nc.scalar.copy(out=srcT[:, c, 125 * k:125 * (k + 1)],
                                           in_=ps_tr[:, :125])
                        else:
                            nc.vector.tensor_copy(out=srcT[:, c, 125 * k:125 * (k + 1)],
                                                  in_=ps_tr[:, :125])
                ps_h = pps.tile([128, SRC_CHUNK], F32, tag="ps_b")
                for c in range(2):
                    nc.tensor.matmul(ps_h, w_bf[:, c, :], srcT[:, c, :],
                                     start=(c == 0), stop=(c == 1))
                h1t = persist.tile([128, SRC_CHUNK], BF16, tag=f"h1t{j}")
                nc.vector.tensor_copy(out=h1t, in_=ps_h)
                h1t_tiles.append(h1t)

        # ================= main loop =================
        with tc.tile_pool(name="mn_bias", bufs=3) as pbias, \
             tc.tile_pool(name="mn_mask", bufs=3) as pmask, \
             tc.tile_pool(name="mn_ms", bufs=3) as pms, \
             tc.tile_pool(name="mn_u", bufs=3) as pu, \
             tc.tile_pool(name="mn_small", bufs=2) as psm, \
             tc.tile_pool(name="mn_ps", bufs=3, space="PSUM") as mps:

            for ti, (r0, rn) in enumerate(DEST_TILES):
                gt = gt_tiles[ti]
                dpart = psm.tile([128, N_SS], F32, tag="dpart")
                npart = psm.tile([128, N_SS], F32, tag="npart")

                for ss in range(N_SS):
                    c0 = ss * SS_W
                    btile = pbias.tile([128, SS_W], F32, tag="btile")
                    nc.sync.dma_start(out=btile[:rn, :],
                                      in_=bias_t[r0:r0 + rn, c0:c0 + SS_W])
                    amask = pmask.tile([128, SS_W], F32, tag="amask")
                    nc.gpsimd.tensor_scalar(
                        out=amask[:rn, :], in0=btile[:rn, :],
                        scalar1=0.0, scalar2=-60000.0,
                        op0=OP.is_le, op1=OP.mult)

                    # [128, 2, 512]: each matmul output sits in its own
                    # PSUM bank (matmul must not cross bank boundaries)
                    ps_e = mps.tile([128, 2, 512], F32, tag="ps_e")
                    for h in range(2):
                        nc.tensor.matmul(
                            ps_e[:rn, h, 0:SRC_CHUNK],
                            gt[:, :rn], h1t_tiles[2 * ss + h],
                            start=True, stop=True)

                    # ms = e + amask (fp16; -60000 mask survives, e fits)
                    ms = pms.tile([128, 2, SRC_CHUNK], FP16, tag="ms")
                    nc.vector.tensor_add(ms[:rn], ps_e[:rn, :, 0:SRC_CHUNK],
                                         amask[:rn, :].rearrange(
                                             "p (b c) -> p b c", b=2))
                    # t = lrelu(ms) = max(0.2*ms, ms)
                    msf = ms[:rn].rearrange("p b c -> p (b c)")
                    t = pms.tile([128, SS_W], FP16, tag="t")
                    nc.vector.scalar_tensor_tensor(
                        out=t[:rn, :], in0=msf, scalar=0.2,
                        in1=msf, op0=OP.mult, op1=OP.max)
                    # u = exp(SCALE * t); denom partial via accum
                    u = pu.tile([128, SS_W], BF16, tag="u")
                    nc.scalar.activation(out=u[:rn, :], in_=t[:rn, :],
                                         func=AF.Exp, scale=SCALE,
                                         accum_out=dpart[:rn, ss:ss + 1])
                    # numer partial: sum(u * ft) via stt accum
                    scrap = pu.tile([128, SS_W], BF16, tag="scrap")
                    nc.vector.scalar_tensor_tensor(
                        out=scrap[:rn, :], in0=u[:rn, :], scalar=0.0,
                        in1=ftbc_bf[:rn, c0:c0 + SS_W],
                        op0=OP.bypass, op1=OP.mult,
                        accum_out=npart[:rn, ss:ss + 1])

                den = psm.tile([128, 1], F32, tag="den")
                nc.vector.tensor_reduce(den[:rn, :], dpart[:rn, :],
                                        axis=mybir.AxisListType.X, op=OP.add)
                num = psm.tile([128, 1], F32, tag="num")
                nc.vector.tensor_reduce(num[:rn, :], npart[:rn, :],
                                        axis=mybir.AxisListType.X, op=OP.add)
                rden = psm.tile([128, 1], F32, tag="rden")
                nc.vector.reciprocal(out=rden[:rn, :], in_=den[:rn, :])
                o = psm.tile([128, 1], F32, tag="o")
                nc.vector.tensor_mul(o[:rn, :], num[:rn, :], rden[:rn, :])
                nc.sync.dma_start(out=out_t[r0:r0 + rn, :], in_=o[:rn, :])

    nc.compile()
    return nc


def _get_nc():
    if "nc" not in _CACHE:
        _CACHE["nc"] = _build_nc()
    return _CACHE["nc"]


def kernel(bias, emb_dest, emb_src, feature_src, W, W2, _trace=False):
    from concourse.bass_utils import run_bass_kernel_spmd

    bias = np.ascontiguousarray(bias, dtype=np.float32)
    emb_dest = np.ascontiguousarray(emb_dest, dtype=np.float32)
    emb_src = np.ascontiguousarray(emb_src, dtype=np.float32)
    ft = np.ascontiguousarray(feature_src, dtype=np.float32)
    W = np.ascontiguousarray(W, dtype=np.float32)
    W2 = np.ascontiguousarray(W2, dtype=np.float32)

    nan_ind = np.isnan(ft.reshape(-1))
    if nan_ind.any():
        # NaN source features: zero the feature and mask out the column
        # (matches reference semantics). Never hit for randn inputs.
        ft = np.where(np.isnan(ft), 0.0, ft)
        bias = np.where(nan_ind.reshape(1, -1), -1.0, bias)

    nc = _get_nc()
    in_maps = []
    for i in range(N_CORES):
        r0 = i * ND
        in_maps.append({
            "bias": bias[r0:r0 + ND],
            "emb_dest": emb_dest[r0:r0 + ND],
            "emb_src": emb_src,
            "feature_src": ft,
            "W": W,
            "W2": W2,
        })
    res = run_bass_kernel_spmd(nc, in_maps, list(range(N_CORES)),
                               trace=_trace)
    out = np.concatenate([res.results[i]["out"] for i in range(N_CORES)], axis=0)
    if _trace:
        return out, res
    return out


# revision 9
# speedup vs baseline: 2.9841x; 2.9841x over previous
"""Trainium2 Bass kernel for nn_AttentionLayer (GAT-style masked attention).

Computes, for full inputs:
    h1 = emb_src @ W                      [8000, 128]
    g  = emb_dest @ (W @ W2)              [10000, 128]
    e  = g @ h1.T                         [10000, 8000]
    s  = lrelu(e, 0.2) * (1/sqrt(128))    masked to -inf where bias <= 0
    att = softmax(s, axis=1)
    out = att @ ft                        [10000, 1]   (ft = nan-cleaned feature_src)

Sharding: N_dest split across 8 NeuronCores (1250 rows each); emb_src /
feature_src / W / W2 replicated. No collectives. Softmax is computed
unnormalized (numer/denom) — no max-subtraction needed since |s| <= ~10.

Per-core device pipeline (per 128-row dest tile x 1000-col src slice):
    PE:     e_psum = gT.T @ h1T                       (bf16 x bf16 -> f32 PSUM)
    GPSIMD: amask  = (bias <= 0) * -1e30              (from streamed bias tile)
    DVE:    ms     = e_psum + amask
    ACT:    t      = Lrelu(SCALE * ms)  [alpha=0.2]
    ACT:    u      = Exp(t)             [accum_out -> denom partial]
    DVE:    ttr u * ft_bcast            [accum_out -> numer partial]
    out = numer / denom
"""
import os
import sys

sys.path.insert(0, "/opt/trn_rl_repo")

import numpy as np

_CACHE = {}

N_DEST, N_SRC, IN_DIM, HID = 10000, 8000, 256, 128
N_CORES = 8
ND = N_DEST // N_CORES            # 1250 dest rows per core
SCALE = float(1.0 / np.sqrt(np.float32(HID)))

# dest tiles per core: 9 x 128 + 98
DEST_TILES = [(i * 128, min(128, ND - i * 128)) for i in range((ND + 127) // 128)]
SRC_CHUNK = 500                   # matmul N (<= 512 = one PSUM bank of f32)
N_SRC_CHUNKS = N_SRC // SRC_CHUNK # 16
SS_W = 2 * SRC_CHUNK              # 1000-col superslice for ACT/DVE ops
N_SS = N_SRC // SS_W              # 8


def _build_nc():
    import concourse.bass as bass
    import concourse.tile as tile
    from concourse import bacc, mybir
    from concourse.masks import make_identity
    from contextlib import ExitStack

    F32 = mybir.dt.float32
    BF16 = mybir.dt.bfloat16
    FP16 = mybir.dt.float16
    AF = mybir.ActivationFunctionType
    OP = mybir.AluOpType

    nc = bacc.Bacc("TRN2", target_bir_lowering=False, debug=False,
                   num_devices=N_CORES)

    bias_t = nc.declare_dram_parameter("bias", [ND, N_SRC], F32, isOutput=False)
    dest_t = nc.declare_dram_parameter("emb_dest", [ND, IN_DIM], F32, isOutput=False)
    src_t = nc.declare_dram_parameter("emb_src", [N_SRC, IN_DIM], F32, isOutput=False)
    ft_t = nc.declare_dram_parameter("feature_src", [N_SRC, 1], F32, isOutput=False)
    w_t = nc.declare_dram_parameter("W", [IN_DIM, HID], F32, isOutput=False)
    w2_t = nc.declare_dram_parameter("W2", [HID, HID], F32, isOutput=False)
    out_t = nc.declare_dram_parameter("out", [ND, 1], F32, isOutput=True)

    with tile.TileContext(nc) as tc, ExitStack() as ctx:
        persist = ctx.enter_context(tc.tile_pool(name="persist", bufs=1))

        ident = persist.tile([128, 128], F32)
        make_identity(nc, ident)

        # ftbc: feature row broadcast across 128 partitions, then bf16
        ft_row = ft_t[:, :].rearrange("s one -> one s")   # [1, 8000] view
        ftbc = persist.tile([128, N_SRC], F32)
        nc.sync.dma_start(out=ftbc, in_=ft_row.to_broadcast([128, N_SRC]))
        ftbc_bf = persist.tile([128, N_SRC], BF16)
        nc.vector.tensor_copy(out=ftbc_bf, in_=ftbc)

        gt_tiles = []
        h1t_tiles = []

        with tc.tile_pool(name="pre_sb", bufs=3) as pre, \
             tc.tile_pool(name="pre_ps", bufs=2, space="PSUM") as pps:

            # ---- W chunks ([K=in_dim sub, M=hid]) natural + bf16
            w_sb = pre.tile([128, 2, HID], F32, tag="w_sb")
            for c in range(2):
                nc.sync.dma_start(out=w_sb[:, c, :], in_=w_t[128 * c:128 * (c + 1), :])
            w_bf = persist.tile([128, 2, HID], BF16)
            nc.vector.tensor_copy(out=w_bf, in_=w_sb)
            w2_sb = pre.tile([128, HID], F32, tag="w2_sb")
            nc.sync.dma_start(out=w2_sb, in_=w2_t[:, :])

            # ---- Wc = W @ W2, stored as lhsT chunks [K=in_dim sub, M=hid] bf16
            wc_bf = persist.tile([128, 2, HID], BF16)
            for c in range(2):
                ps_tr = pps.tile([128, 128], F32, tag="ps_a")
                nc.tensor.transpose(ps_tr, w_sb[:, c, :], ident)    # [hid, in_sub]
                wTc = pre.tile([128, 128], F32, tag="wTc")
                nc.scalar.copy(out=wTc, in_=ps_tr)
                ps_mm = pps.tile([128, HID], F32, tag="ps_b")
                nc.tensor.matmul(ps_mm, wTc, w2_sb, start=True, stop=True)
                nc.scalar.copy(out=wc_bf[:, c, :], in_=ps_mm)

            # ---- emb_dest -> destT (bf16, [in_sub, 2, dest]) -> gT tiles
            destT = pre.tile([128, 2, ND], BF16, tag="destT")
            for (r0, rn) in DEST_TILES:
                ed = pre.tile([128, IN_DIM], F32, tag="ed")
                nc.sync.dma_start(out=ed[:rn, :], in_=dest_t[r0:r0 + rn, :])
                for c in range(2):
                    ps_tr = pps.tile([128, 128], F32, tag="ps_a")
                    nc.tensor.transpose(ps_tr[:, :rn], ed[:rn, 128 * c:128 * (c + 1)],
                                        ident[:rn, :rn])
                    if c == 0:
                        nc.scalar.copy(out=destT[:, c, r0:r0 + rn], in_=ps_tr[:, :rn])
                    else:
                        nc.vector.tensor_copy(out=destT[:, c, r0:r0 + rn], in_=ps_tr[:, :rn])
            for ti, (r0, rn) in enumerate(DEST_TILES):
                ps_g = pps.tile([128, 128], F32, tag="ps_b")
                for c in range(2):
                    nc.tensor.matmul(ps_g[:, :rn], wc_bf[:, c, :],
                                     destT[:, c, r0:r0 + rn],
                                     start=(c == 0), stop=(c == 1))
                gt = persist.tile([128, 128], BF16, tag=f"gt{ti}")
                nc.scalar.copy(out=gt[:, :rn], in_=ps_g[:, :rn])
                gt_tiles.append(gt)

            # ---- emb_src -> srcT chunks -> h1T chunk tiles
            for j in range(N_SRC_CHUNKS):
                srcT = pre.tile([128, 2, SRC_CHUNK], BF16, tag="srcT")
                for k in range(4):                       # 4 x 125 src rows
                    s0 = j * SRC_CHUNK + k * 125
                    es = pre.tile([125, IN_DIM], F32, tag="es")
                    nc.sync.dma_start(out=es, in_=src_t[s0:s0 + 125, :])
                    for c in range(2):
                        ps_tr = pps.tile([128, 128], F32, tag="ps_a")
                        nc.tensor.transpose(ps_tr[:, :125],
                                            es[:, 128 * c:128 * (c + 1)],
                                            ident[:125, :125])
                        if (k + c) % 2 == 0:
                            nc.scalar.copy(out=srcT[:, c, 125 * k:125 * (k + 1)],
                                           in_=ps_tr[:, :125])
                        else:
                            nc.vector.tensor_copy(out=srcT[:, c, 125 * k:125 * (k + 1)],
                                                  in_=ps_tr[:, :125])
                ps_h = pps.tile([128, SRC_CHUNK], F32, tag="ps_b")
                for c in range(2):
                    nc.tensor.matmul(ps_h, w_bf[:, c, :], srcT[:, c, :],
                                     start=(c == 0), stop=(c == 1))
                h1t = persist.tile([128, SRC_CHUNK], BF16, tag=f"h1t{j}")
                nc.vector.tensor_copy(out=h1t, in_=ps_h)
                h1t_tiles.append(h1t)

        # ================= main loop =================
        with tc.tile_pool(name="mn_bias", bufs=3) as pbias, \
             tc.tile_pool(name="mn_mask", bufs=3) as pmask, \
             tc.tile_pool(name="mn_ms", bufs=3) as pms, \
             tc.tile_pool(name="mn_u", bufs=3) as pu, \
             tc.tile_pool(name="mn_small", bufs=2) as psm, \
             tc.tile_pool(name="mn_ps", bufs=3, space="PSUM") as mps:

            for ti, (r0, rn) in enumerate(DEST_TILES):
                gt = gt_tiles[ti]
                dpart = psm.tile([128, N_SS], F32, tag="dpart")
                npart = psm.tile([128, N_SS], F32, tag="npart")

                for ss in range(N_SS):
                    c0 = ss * SS_W
                    btile = pbias.tile([128, SS_W], F32, tag="btile")
                    nc.sync.dma_start(out=btile[:rn, :],
                                      in_=bias_t[r0:r0 + rn, c0:c0 + SS_W])
                    amask = pmask.tile([128, SS_W], F32, tag="amask")
                    nc.vector.tensor_scalar(
                        out=amask[:rn, :], in0=btile[:rn, :],
                        scalar1=0.0, scalar2=-60000.0,
                        op0=OP.is_le, op1=OP.mult)

                    # [128, 2, 512]: each matmul output sits in its own
                    # PSUM bank (matmul must not cross bank boundaries)
                    ps_e = mps.tile([128, 2, 512], F32, tag="ps_e")
                    for h in range(2):
                        nc.tensor.matmul(
                            ps_e[:rn, h, 0:SRC_CHUNK],
                            gt[:, :rn], h1t_tiles[2 * ss + h],
                            start=True, stop=True)

                    # ms = e + amask (fp16; -60000 mask survives, e fits)
                    ms = pms.tile([128, 2, SRC_CHUNK], FP16, tag="ms")
                    nc.vector.tensor_add(ms[:rn], ps_e[:rn, :, 0:SRC_CHUNK],
                                         amask[:rn, :].rearrange(
                                             "p (b c) -> p b c", b=2))
                    # t = lrelu(ms) = max(0.2*ms, ms)
                    msf = ms[:rn].rearrange("p b c -> p (b c)")
                    t0 = pms.tile([128, SS_W], FP16, tag="t0")
                    nc.vector.tensor_scalar_mul(t0[:rn, :], msf, 0.2)
                    t = pms.tile([128, SS_W], FP16, tag="t")
                    nc.vector.tensor_max(t[:rn, :], msf, t0[:rn, :])
                    # u = exp(SCALE * t); denom partial via accum
                    u = pu.tile([128, SS_W], BF16, tag="u")
                    nc.scalar.activation(out=u[:rn, :], in_=t[:rn, :],
                                         func=AF.Exp, scale=SCALE,
                                         accum_out=dpart[:rn, ss:ss + 1])
                    # numer partial: sum(u * ft)
                    prod = pu.tile([128, SS_W], BF16, tag="prod")
                    nc.vector.tensor_mul(prod[:rn, :], u[:rn, :],
                                         ftbc_bf[:rn, c0:c0 + SS_W])
                    nc.vector.tensor_reduce(
                        npart[:rn, ss:ss + 1], prod[:rn, :],
                        axis=mybir.AxisListType.X, op=OP.add)

                den = psm.tile([128, 1], F32, tag="den")
                nc.vector.tensor_reduce(den[:rn, :], dpart[:rn, :],
                                        axis=mybir.AxisListType.X, op=OP.add)
                num = psm.tile([128, 1], F32, tag="num")
                nc.vector.tensor_reduce(num[:rn, :], npart[:rn, :],
                                        axis=mybir.AxisListType.X, op=OP.add)
                rden = psm.tile([128, 1], F32, tag="rden")
                nc.vector.reciprocal(out=rden[:rn, :], in_=den[:rn, :])
                o = psm.tile([128, 1], F32, tag="o")
                nc.vector.tensor_mul(o[:rn, :], num[:rn, :], rden[:rn, :])
                nc.sync.dma_start(out=out_t[r0:r0 + rn, :], in_=o[:rn, :])

    nc.compile()
    return nc


def _get_nc():
    if "nc" not in _CACHE:
        _CACHE["nc"] = _build_nc()
    return _CACHE["nc"]


def kernel(bias, emb_dest, emb_src, feature_src, W, W2, _trace=False):
    from concourse.bass_utils import run_bass_kernel_spmd

    bias = np.ascontiguousarray(bias, dtype=np.float32)
    emb_dest = np.ascontiguousarray(emb_dest, dtype=np.float32)
    emb_src = np.ascontiguousarray(emb_src, dtype=np.float32)
    ft = np.ascontiguousarray(feature_src, dtype=np.float32)
    W = np.ascontiguousarray(W, dtype=np.float32)
    W2 = np.ascontiguousarray(W2, dtype=np.float32)

    nan_ind = np.isnan(ft.reshape(-1))
    if nan_ind.any():
        # NaN source features: zero the feature and mask out the column
        # (matches reference semantics). Never hit for randn inputs.
        ft = np.where(np.isnan(ft), 0.0, ft)
        bias = np.where(nan_ind.reshape(1, -1), -1.0, bias)

    nc = _get_nc()
    in_maps = []
    for i in range(N_CORES):
        r0 = i * ND
        in_maps.append({
            "bias": bias[r0:r0 + ND],
            "emb_dest": emb_dest[r0:r0 + ND],
            "emb_src": emb_src,
            "feature_src": ft,
            "W": W,
            "W2": W2,
        })
    res = run_bass_kernel_spmd(nc, in_maps, list(range(N_CORES)),
                               trace=_trace)
    out = np.concatenate([res.results[i]["out"] for i in range(N_CORES)], axis=0)
    if _trace:
        return out, res
    return out


# revision 10
# speedup vs baseline: 3.3406x; 1.1195x over previous
"""Trainium2 Bass kernel for nn_AttentionLayer (GAT-style masked attention).

Computes, for full inputs:
    h1 = emb_src @ W                      [8000, 128]
    g  = emb_dest @ (W @ W2)              [10000, 128]
    e  = g @ h1.T                         [10000, 8000]
    s  = lrelu(e, 0.2) * (1/sqrt(128))    masked to -inf where bias <= 0
    att = softmax(s, axis=1)
    out = att @ ft                        [10000, 1]   (ft = nan-cleaned feature_src)

Sharding: N_dest split across 8 NeuronCores (1250 rows each); emb_src /
feature_src / W / W2 replicated. No collectives. Softmax is computed
unnormalized (numer/denom) — no max-subtraction needed since |s| <= ~10.

Per-core device pipeline (per 128-row dest tile x 1000-col src slice):
    PE:     e_psum = gT.T @ h1T                       (bf16 x bf16 -> f32 PSUM)
    GPSIMD: amask  = (bias <= 0) * -1e30              (from streamed bias tile)
    DVE:    ms     = e_psum + amask
    ACT:    t      = Lrelu(SCALE * ms)  [alpha=0.2]
    ACT:    u      = Exp(t)             [accum_out -> denom partial]
    DVE:    ttr u * ft_bcast            [accum_out -> numer partial]
    out = numer / denom
"""
import os
import sys

sys.path.insert(0, "/opt/trn_rl_repo")

import numpy as np

_CACHE = {}

N_DEST, N_SRC, IN_DIM, HID = 10000, 8000, 256, 128
N_CORES = 8
ND = N_DEST // N_CORES            # 1250 dest rows per core
SCALE = float(1.0 / np.sqrt(np.float32(HID)))

# dest tiles per core: 9 x 128 + 98
DEST_TILES = [(i * 128, min(128, ND - i * 128)) for i in range((ND + 127) // 128)]
SRC_CHUNK = 500                   # matmul N (<= 512 = one PSUM bank of f32)
N_SRC_CHUNKS = N_SRC // SRC_CHUNK # 16
SS_W = 2 * SRC_CHUNK              # 1000-col superslice for ACT/DVE ops
N_SS = N_SRC // SS_W              # 8


def _build_nc():
    import concourse.bass as bass
    import concourse.tile as tile
    from concourse import bacc, mybir
    from concourse.masks import make_identity
    from contextlib import ExitStack

    F32 = mybir.dt.float32
    BF16 = mybir.dt.bfloat16
    FP16 = mybir.dt.float16
    AF = mybir.ActivationFunctionType
    OP = mybir.AluOpType

    nc = bacc.Bacc("TRN2", target_bir_lowering=False, debug=False,
                   num_devices=N_CORES)

    bias_t = nc.declare_dram_parameter("bias", [ND, N_SRC], F32, isOutput=False)
    dest_t = nc.declare_dram_parameter("emb_dest", [ND, IN_DIM], F32, isOutput=False)
    src_t = nc.declare_dram_parameter("emb_src", [N_SRC, IN_DIM], F32, isOutput=False)
    ft_t = nc.declare_dram_parameter("feature_src", [N_SRC, 1], F32, isOutput=False)
    w_t = nc.declare_dram_parameter("W", [IN_DIM, HID], F32, isOutput=False)
    w2_t = nc.declare_dram_parameter("W2", [HID, HID], F32, isOutput=False)
    out_t = nc.declare_dram_parameter("out", [ND, 1], F32, isOutput=True)

    with tile.TileContext(nc) as tc, ExitStack() as ctx:
        persist = ctx.enter_context(tc.tile_pool(name="persist", bufs=1))

        ident = persist.tile([128, 128], F32)
        make_identity(nc, ident)

        # ftbc: feature row broadcast across 128 partitions, then bf16
        ft_row = ft_t[:, :].rearrange("s one -> one s")   # [1, 8000] view
        ftbc = persist.tile([128, N_SRC], F32)
        nc.sync.dma_start(out=ftbc, in_=ft_row.to_broadcast([128, N_SRC]))
        ftbc_bf = persist.tile([128, N_SRC], BF16)
        nc.vector.tensor_copy(out=ftbc_bf, in_=ftbc)

        gt_tiles = []
        h1t_tiles = []

        with tc.tile_pool(name="pre_sb", bufs=3) as pre, \
             tc.tile_pool(name="pre_ps", bufs=2, space="PSUM") as pps:

            # ---- W chunks ([K=in_dim sub, M=hid]) natural + bf16
            w_sb = pre.tile([128, 2, HID], F32, tag="w_sb")
            for c in range(2):
                nc.sync.dma_start(out=w_sb[:, c, :], in_=w_t[128 * c:128 * (c + 1), :])
            w_bf = persist.tile([128, 2, HID], BF16)
            nc.vector.tensor_copy(out=w_bf, in_=w_sb)
            w2_sb = pre.tile([128, HID], F32, tag="w2_sb")
            nc.sync.dma_start(out=w2_sb, in_=w2_t[:, :])

            # ---- Wc = W @ W2, stored as lhsT chunks [K=in_dim sub, M=hid] bf16
            wc_bf = persist.tile([128, 2, HID], BF16)
            for c in range(2):
                ps_tr = pps.tile([128, 128], F32, tag="ps_a")
                nc.tensor.transpose(ps_tr, w_sb[:, c, :], ident)    # [hid, in_sub]
                wTc = pre.tile([128, 128], F32, tag="wTc")
                nc.scalar.copy(out=wTc, in_=ps_tr)
                ps_mm = pps.tile([128, HID], F32, tag="ps_b")
                nc.tensor.matmul(ps_mm, wTc, w2_sb, start=True, stop=True)
                nc.scalar.copy(out=wc_bf[:, c, :], in_=ps_mm)

            # ---- emb_dest -> destT (bf16, [in_sub, 2, dest]) -> gT tiles
            destT = pre.tile([128, 2, ND], BF16, tag="destT")
            for (r0, rn) in DEST_TILES:
                ed = pre.tile([128, IN_DIM], F32, tag="ed")
                nc.sync.dma_start(out=ed[:rn, :], in_=dest_t[r0:r0 + rn, :])
                for c in range(2):
                    ps_tr = pps.tile([128, 128], F32, tag="ps_a")
                    nc.tensor.transpose(ps_tr[:, :rn], ed[:rn, 128 * c:128 * (c + 1)],
                                        ident[:rn, :rn])
                    if c == 0:
                        nc.scalar.copy(out=destT[:, c, r0:r0 + rn], in_=ps_tr[:, :rn])
                    else:
                        nc.vector.tensor_copy(out=destT[:, c, r0:r0 + rn], in_=ps_tr[:, :rn])
            for ti, (r0, rn) in enumerate(DEST_TILES):
                ps_g = pps.tile([128, 128], F32, tag="ps_b")
                for c in range(2):
                    nc.tensor.matmul(ps_g[:, :rn], wc_bf[:, c, :],
                                     destT[:, c, r0:r0 + rn],
                                     start=(c == 0), stop=(c == 1))
                gt = persist.tile([128, 128], BF16, tag=f"gt{ti}")
                nc.scalar.copy(out=gt[:, :rn], in_=ps_g[:, :rn])
                gt_tiles.append(gt)

            # ---- emb_src -> srcT chunks -> h1T chunk tiles
            for j in range(N_SRC_CHUNKS):
                srcT = pre.tile([128, 2, SRC_CHUNK], BF16, tag="srcT")
                for k in range(4):                       # 4 x 125 src rows
                    s0 = j * SRC_CHUNK + k * 125
                    es = pre.tile([125, IN_DIM], F32, tag="es")
                    nc.sync.dma_start(out=es, in_=src_t[s0:s0 + 125, :])
                    for c in range(2):
                        ps_tr = pps.tile([128, 128], F32, tag="ps_a")
                        nc.tensor.transpose(ps_tr[:, :125],
                                            es[:, 128 * c:128 * (c + 1)],
                                            ident[:125, :125])
                        if (k + c) % 2 == 0:
                            nc.scalar.copy(out=srcT[:, c, 125 * k:125 * (k + 1)],
                                           in_=ps_tr[:, :125])
                        else:
                            nc.vector.tensor_copy(out=srcT[:, c, 125 * k:125 * (k + 1)],
                                                  in_=ps_tr[:, :125])
                ps_h = pps.tile([128, SRC_CHUNK], F32, tag="ps_b")
                for c in range(2):
                    nc.tensor.matmul(ps_h, w_bf[:, c, :], srcT[:, c, :],
                                     start=(c == 0), stop=(c == 1))
                h1t = persist.tile([128, SRC_CHUNK], BF16, tag=f"h1t{j}")
                nc.vector.tensor_copy(out=h1t, in_=ps_h)
                h1t_tiles.append(h1t)

        # ================= main loop =================
        with tc.tile_pool(name="mn_bias", bufs=3) as pbias, \
             tc.tile_pool(name="mn_mask", bufs=3) as pmask, \
             tc.tile_pool(name="mn_ms", bufs=3) as pms, \
             tc.tile_pool(name="mn_u", bufs=3) as pu, \
             tc.tile_pool(name="mn_small", bufs=2) as psm, \
             tc.tile_pool(name="mn_ps", bufs=3, space="PSUM") as mps:

            for ti, (r0, rn) in enumerate(DEST_TILES):
                gt = gt_tiles[ti]
                dpart = psm.tile([128, N_SS], F32, tag="dpart")
                npart = psm.tile([128, N_SS], F32, tag="npart")

                for ss in range(N_SS):
                    c0 = ss * SS_W
                    btile = pbias.tile([128, SS_W], F32, tag="btile")
                    nc.sync.dma_start(out=btile[:rn, :],
                                      in_=bias_t[r0:r0 + rn, c0:c0 + SS_W])
                    amask = pmask.tile([128, SS_W], F32, tag="amask")
                    nc.vector.tensor_scalar(
                        out=amask[:rn, :], in0=btile[:rn, :],
                        scalar1=0.0, scalar2=-60000.0,
                        op0=OP.is_le, op1=OP.mult)

                    # [128, 2, 512]: each matmul output sits in its own
                    # PSUM bank (matmul must not cross bank boundaries)
                    ps_e = mps.tile([128, 2, 512], F32, tag="ps_e")
                    for h in range(2):
                        nc.tensor.matmul(
                            ps_e[:rn, h, 0:SRC_CHUNK],
                            gt[:, :rn], h1t_tiles[2 * ss + h],
                            start=True, stop=True)

                    # ms = e + amask (fp16; -60000 mask survives, e fits)
                    ms = pms.tile([128, 2, SRC_CHUNK], FP16, tag="ms")
                    nc.vector.tensor_add(ms[:rn], ps_e[:rn, :, 0:SRC_CHUNK],
                                         amask[:rn, :].rearrange(
                                             "p (b c) -> p b c", b=2))
                    # t = lrelu(ms) = max(0.2*ms, ms)
                    msf = ms[:rn].rearrange("p b c -> p (b c)")
                    t0 = pms.tile([128, SS_W], FP16, tag="t0")
                    nc.vector.tensor_scalar_mul(t0[:rn, :], msf, 0.2)
                    t = pms.tile([128, SS_W], FP16, tag="t")
                    nc.vector.tensor_max(t[:rn, :], msf, t0[:rn, :])
                    # u = exp(SCALE * t); denom partial via accum
                    u = pu.tile([128, SS_W], BF16, tag="u")
                    nc.scalar.activation(out=u[:rn, :], in_=t[:rn, :],
                                         func=AF.Exp, scale=SCALE,
                                         accum_out=dpart[:rn, ss:ss + 1])
                    # numer partial: sum(u * ft); row-sum rides ACT accum
                    prod = pu.tile([128, SS_W], BF16, tag="prod")
                    nc.vector.tensor_mul(prod[:rn, :], u[:rn, :],
                                         ftbc_bf[:rn, c0:c0 + SS_W])
                    scrap = pu.tile([128, SS_W], BF16, tag="scrap")
                    nc.scalar.activation(out=scrap[:rn, :], in_=prod[:rn, :],
                                         func=AF.Copy,
                                         accum_out=npart[:rn, ss:ss + 1])

                den = psm.tile([128, 1], F32, tag="den")
                nc.vector.tensor_reduce(den[:rn, :], dpart[:rn, :],
                                        axis=mybir.AxisListType.X, op=OP.add)
                num = psm.tile([128, 1], F32, tag="num")
                nc.vector.tensor_reduce(num[:rn, :], npart[:rn, :],
                                        axis=mybir.AxisListType.X, op=OP.add)
                rden = psm.tile([128, 1], F32, tag="rden")
                nc.vector.reciprocal(out=rden[:rn, :], in_=den[:rn, :])
                o = psm.tile([128, 1], F32, tag="o")
                nc.vector.tensor_mul(o[:rn, :], num[:rn, :], rden[:rn, :])
                nc.sync.dma_start(out=out_t[r0:r0 + rn, :], in_=o[:rn, :])

    nc.compile()
    return nc


def _get_nc():
    if "nc" not in _CACHE:
        _CACHE["nc"] = _build_nc()
    return _CACHE["nc"]


def kernel(bias, emb_dest, emb_src, feature_src, W, W2, _trace=False):
    from concourse.bass_utils import run_bass_kernel_spmd

    bias = np.ascontiguousarray(bias, dtype=np.float32)
    emb_dest = np.ascontiguousarray(emb_dest, dtype=np.float32)
    emb_src = np.ascontiguousarray(emb_src, dtype=np.float32)
    ft = np.ascontiguousarray(feature_src, dtype=np.float32)
    W = np.ascontiguousarray(W, dtype=np.float32)
    W2 = np.ascontiguousarray(W2, dtype=np.float32)

    nan_ind = np.isnan(ft.reshape(-1))
    if nan_ind.any():
        # NaN source features: zero the feature and mask out the column
        # (matches reference semantics). Never hit for randn inputs.
        ft = np.where(np.isnan(ft), 0.0, ft)
        bias = np.where(nan_ind.reshape(1, -1), -1.0, bias)

    nc = _get_nc()
    in_maps = []
    for i in range(N_CORES):
        r0 = i * ND
        in_maps.append({
            "bias": bias[r0:r0 + ND],
            "emb_dest": emb_dest[r0:r0 + ND],
            "emb_src": emb_src,
            "feature_src": ft,
            "W": W,
            "W2": W2,
        })
    res = run_bass_kernel_spmd(nc, in_maps, list(range(N_CORES)),
                               trace=_trace)
    out = np.concatenate([res.results[i]["out"] for i in range(N_CORES)], axis=0)
    if _trace:
        return out, res
    return out


# revision 11
# speedup vs baseline: 3.6535x; 1.0937x over previous
"""Trainium2 Bass kernel for nn_AttentionLayer (GAT-style masked attention).

Computes, for full inputs:
    h1 = emb_src @ W                      [8000, 128]
    g  = emb_dest @ (W @ W2)              [10000, 128]
    e  = g @ h1.T                         [10000, 8000]
    s  = lrelu(e, 0.2) * (1/sqrt(128))    masked to -inf where bias <= 0
    att = softmax(s, axis=1)
    out = att @ ft                        [10000, 1]   (ft = nan-cleaned feature_src)

Sharding: N_dest split across 8 NeuronCores (1250 rows each); emb_src /
feature_src / W / W2 replicated. No collectives. Softmax is computed
unnormalized (numer/denom) — no max-subtraction needed since |s| <= ~10.

Per-core device pipeline (per 128-row dest tile x 1000-col src slice):
    PE:     e_psum = gT.T @ h1T                       (bf16 x bf16 -> f32 PSUM)
    GPSIMD: amask  = (bias <= 0) * -1e30              (from streamed bias tile)
    DVE:    ms     = e_psum + amask
    ACT:    t      = Lrelu(SCALE * ms)  [alpha=0.2]
    ACT:    u      = Exp(t)             [accum_out -> denom partial]
    DVE:    ttr u * ft_bcast            [accum_out -> numer partial]
    out = numer / denom
"""
import os
import sys

sys.path.insert(0, "/opt/trn_rl_repo")

import numpy as np

_CACHE = {}

N_DEST, N_SRC, IN_DIM, HID = 10000, 8000, 256, 128
N_CORES = 8
ND = N_DEST // N_CORES            # 1250 dest rows per core
SCALE = float(1.0 / np.sqrt(np.float32(HID)))

# dest tiles per core: 9 x 128 + 98
DEST_TILES = [(i * 128, min(128, ND - i * 128)) for i in range((ND + 127) // 128)]
SRC_CHUNK = 500                   # matmul N (<= 512 = one PSUM bank of f32)
N_SRC_CHUNKS = N_SRC // SRC_CHUNK # 16
SS_W = 2 * SRC_CHUNK              # 1000-col superslice for ACT/DVE ops
N_SS = N_SRC // SS_W              # 8


def _build_nc():
    import concourse.bass as bass
    import concourse.tile as tile
    from concourse import bacc, mybir
    from concourse.masks import make_identity
    from contextlib import ExitStack

    F32 = mybir.dt.float32
    BF16 = mybir.dt.bfloat16
    FP16 = mybir.dt.float16
    AF = mybir.ActivationFunctionType
    OP = mybir.AluOpType

    nc = bacc.Bacc("TRN2", target_bir_lowering=False, debug=False,
                   num_devices=N_CORES)

    bias_t = nc.declare_dram_parameter("bias", [ND, N_SRC], F32, isOutput=False)
    dest_t = nc.declare_dram_parameter("emb_dest", [ND, IN_DIM], F32, isOutput=False)
    src_t = nc.declare_dram_parameter("emb_src", [N_SRC, IN_DIM], F32, isOutput=False)
    ft_t = nc.declare_dram_parameter("feature_src", [N_SRC, 1], F32, isOutput=False)
    w_t = nc.declare_dram_parameter("W", [IN_DIM, HID], F32, isOutput=False)
    w2_t = nc.declare_dram_parameter("W2", [HID, HID], F32, isOutput=False)
    out_t = nc.declare_dram_parameter("out", [ND, 1], F32, isOutput=True)

    with tile.TileContext(nc) as tc, ExitStack() as ctx:
        persist = ctx.enter_context(tc.tile_pool(name="persist", bufs=1))

        ident = persist.tile([128, 128], F32)
        make_identity(nc, ident)

        # ftbc: feature row broadcast across 128 partitions, then bf16
        ft_row = ft_t[:, :].rearrange("s one -> one s")   # [1, 8000] view
        ftbc = persist.tile([128, N_SRC], F32)
        nc.sync.dma_start(out=ftbc, in_=ft_row.to_broadcast([128, N_SRC]))
        ftbc_bf = persist.tile([128, N_SRC], BF16)
        nc.vector.tensor_copy(out=ftbc_bf, in_=ftbc)

        gt_tiles = []
        h1t_tiles = []

        with tc.tile_pool(name="pre_sb", bufs=3) as pre, \
             tc.tile_pool(name="pre_ps", bufs=2, space="PSUM") as pps:

            # ---- W chunks ([K=in_dim sub, M=hid]) natural + bf16
            w_sb = pre.tile([128, 2, HID], F32, tag="w_sb")
            for c in range(2):
                nc.sync.dma_start(out=w_sb[:, c, :], in_=w_t[128 * c:128 * (c + 1), :])
            w_bf = persist.tile([128, 2, HID], BF16)
            nc.vector.tensor_copy(out=w_bf, in_=w_sb)
            w2_sb = pre.tile([128, HID], F32, tag="w2_sb")
            nc.sync.dma_start(out=w2_sb, in_=w2_t[:, :])

            # ---- Wc = W @ W2, stored as lhsT chunks [K=in_dim sub, M=hid] bf16
            wc_bf = persist.tile([128, 2, HID], BF16)
            for c in range(2):
                ps_tr = pps.tile([128, 128], F32, tag="ps_a")
                nc.tensor.transpose(ps_tr, w_sb[:, c, :], ident)    # [hid, in_sub]
                wTc = pre.tile([128, 128], F32, tag="wTc")
                nc.scalar.copy(out=wTc, in_=ps_tr)
                ps_mm = pps.tile([128, HID], F32, tag="ps_b")
                nc.tensor.matmul(ps_mm, wTc, w2_sb, start=True, stop=True)
                nc.scalar.copy(out=wc_bf[:, c, :], in_=ps_mm)

            # ---- emb_dest -> destT (bf16, [in_sub, 2, dest]) -> gT tiles
            destT = pre.tile([128, 2, ND], BF16, tag="destT")
            for (r0, rn) in DEST_TILES:
                ed = pre.tile([128, IN_DIM], F32, tag="ed")
                nc.sync.dma_start(out=ed[:rn, :], in_=dest_t[r0:r0 + rn, :])
                for c in range(2):
                    ps_tr = pps.tile([128, 128], F32, tag="ps_a")
                    nc.tensor.transpose(ps_tr[:, :rn], ed[:rn, 128 * c:128 * (c + 1)],
                                        ident[:rn, :rn])
                    if c == 0:
                        nc.scalar.copy(out=destT[:, c, r0:r0 + rn], in_=ps_tr[:, :rn])
                    else:
                        nc.vector.tensor_copy(out=destT[:, c, r0:r0 + rn], in_=ps_tr[:, :rn])
            for ti, (r0, rn) in enumerate(DEST_TILES):
                ps_g = pps.tile([128, 128], F32, tag="ps_b")
                for c in range(2):
                    nc.tensor.matmul(ps_g[:, :rn], wc_bf[:, c, :],
                                     destT[:, c, r0:r0 + rn],
                                     start=(c == 0), stop=(c == 1))
                gt = persist.tile([128, 128], BF16, tag=f"gt{ti}")
                nc.scalar.copy(out=gt[:, :rn], in_=ps_g[:, :rn])
                gt_tiles.append(gt)

            # ---- emb_src -> srcT chunks -> h1T chunk tiles
            for j in range(N_SRC_CHUNKS):
                srcT = pre.tile([128, 2, SRC_CHUNK], BF16, tag="srcT")
                for k in range(4):                       # 4 x 125 src rows
                    s0 = j * SRC_CHUNK + k * 125
                    es = pre.tile([125, IN_DIM], F32, tag="es")
                    nc.sync.dma_start(out=es, in_=src_t[s0:s0 + 125, :])
                    for c in range(2):
                        ps_tr = pps.tile([128, 128], F32, tag="ps_a")
                        nc.tensor.transpose(ps_tr[:, :125],
                                            es[:, 128 * c:128 * (c + 1)],
                                            ident[:125, :125])
                        if (k + c) % 2 == 0:
                            nc.scalar.copy(out=srcT[:, c, 125 * k:125 * (k + 1)],
                                           in_=ps_tr[:, :125])
                        else:
                            nc.vector.tensor_copy(out=srcT[:, c, 125 * k:125 * (k + 1)],
                                                  in_=ps_tr[:, :125])
                ps_h = pps.tile([128, SRC_CHUNK], F32, tag="ps_b")
                for c in range(2):
                    nc.tensor.matmul(ps_h, w_bf[:, c, :], srcT[:, c, :],
                                     start=(c == 0), stop=(c == 1))
                h1t = persist.tile([128, SRC_CHUNK], BF16, tag=f"h1t{j}")
                nc.vector.tensor_copy(out=h1t, in_=ps_h)
                h1t_tiles.append(h1t)

        # ================= main loop =================
        with tc.tile_pool(name="mn_bias", bufs=3) as pbias, \
             tc.tile_pool(name="mn_mask", bufs=3) as pmask, \
             tc.tile_pool(name="mn_ms", bufs=3) as pms, \
             tc.tile_pool(name="mn_u", bufs=3) as pu, \
             tc.tile_pool(name="mn_small", bufs=2) as psm, \
             tc.tile_pool(name="mn_ps", bufs=3, space="PSUM") as mps:

            for ti, (r0, rn) in enumerate(DEST_TILES):
                gt = gt_tiles[ti]
                dpart = psm.tile([128, N_SS // 2], F32, tag="dpart")
                npart = psm.tile([128, N_SS // 2], F32, tag="npart")

                for p in range(N_SS // 2):
                    c0 = p * 2 * SS_W
                    btile = pbias.tile([128, 2 * SS_W], F32, tag="btile")
                    nc.sync.dma_start(out=btile[:rn, :],
                                      in_=bias_t[r0:r0 + rn, c0:c0 + 2 * SS_W])
                    amask = pmask.tile([128, 2 * SS_W], F32, tag="amask")
                    nc.vector.tensor_scalar(
                        out=amask[:rn, :], in0=btile[:rn, :],
                        scalar1=0.0, scalar2=-60000.0,
                        op0=OP.is_le, op1=OP.mult)

                    ms = pms.tile([128, 4, SRC_CHUNK], FP16, tag="ms")
                    for h in range(2):
                        ps_e = mps.tile([128, 2, 512], F32, tag="ps_e")
                        for q in range(2):
                            nc.tensor.matmul(
                                ps_e[:rn, q, 0:SRC_CHUNK],
                                gt[:, :rn], h1t_tiles[4 * p + 2 * h + q],
                                start=True, stop=True)
                        nc.vector.tensor_add(
                            ms[:rn, 2 * h:2 * h + 2, :],
                            ps_e[:rn, :, 0:SRC_CHUNK],
                            amask[:rn, h * SS_W:(h + 1) * SS_W].rearrange(
                                "p (b c) -> p b c", b=2))

                    msf = ms[:rn].rearrange("p b c -> p (b c)")
                    t0 = pms.tile([128, 2 * SS_W], FP16, tag="t0")
                    nc.vector.tensor_scalar_mul(t0[:rn, :], msf, 0.2)
                    t = pms.tile([128, 2 * SS_W], FP16, tag="t")
                    nc.vector.tensor_max(t[:rn, :], msf, t0[:rn, :])
                    u = pu.tile([128, 2 * SS_W], BF16, tag="u")
                    nc.scalar.activation(out=u[:rn, :], in_=t[:rn, :],
                                         func=AF.Exp, scale=SCALE,
                                         accum_out=dpart[:rn, p:p + 1])
                    prod = pu.tile([128, 2 * SS_W], BF16, tag="prod")
                    nc.vector.tensor_mul(prod[:rn, :], u[:rn, :],
                                         ftbc_bf[:rn, c0:c0 + 2 * SS_W])
                    scrap = pu.tile([128, 2 * SS_W], BF16, tag="scrap")
                    nc.scalar.activation(out=scrap[:rn, :], in_=prod[:rn, :],
                                         func=AF.Copy,
                                         accum_out=npart[:rn, p:p + 1])

                den = psm.tile([128, 1], F32, tag="den")
                nc.vector.tensor_reduce(den[:rn, :], dpart[:rn, :],
                                        axis=mybir.AxisListType.X, op=OP.add)
                num = psm.tile([128, 1], F32, tag="num")
                nc.vector.tensor_reduce(num[:rn, :], npart[:rn, :],
                                        axis=mybir.AxisListType.X, op=OP.add)
                rden = psm.tile([128, 1], F32, tag="rden")
                nc.vector.reciprocal(out=rden[:rn, :], in_=den[:rn, :])
                o = psm.tile([128, 1], F32, tag="o")
                nc.vector.tensor_mul(o[:rn, :], num[:rn, :], rden[:rn, :])
                nc.sync.dma_start(out=out_t[r0:r0 + rn, :], in_=o[:rn, :])

    nc.compile()
    return nc


def _get_nc():
    if "nc" not in _CACHE:
        _CACHE["nc"] = _build_nc()
    return _CACHE["nc"]


def kernel(bias, emb_dest, emb_src, feature_src, W, W2, _trace=False):
    from concourse.bass_utils import run_bass_kernel_spmd

    bias = np.ascontiguousarray(bias, dtype=np.float32)
    emb_dest = np.ascontiguousarray(emb_dest, dtype=np.float32)
    emb_src = np.ascontiguousarray(emb_src, dtype=np.float32)
    ft = np.ascontiguousarray(feature_src, dtype=np.float32)
    W = np.ascontiguousarray(W, dtype=np.float32)
    W2 = np.ascontiguousarray(W2, dtype=np.float32)

    nan_ind = np.isnan(ft.reshape(-1))
    if nan_ind.any():
        # NaN source features: zero the feature and mask out the column
        # (matches reference semantics). Never hit for randn inputs.
        ft = np.where(np.isnan(ft), 0.0, ft)
        bias = np.where(nan_ind.reshape(1, -1), -1.0, bias)

    nc = _get_nc()
    in_maps = []
    for i in range(N_CORES):
        r0 = i * ND
        in_maps.append({
            "bias": bias[r0:r0 + ND],
            "emb_dest": emb_dest[r0:r0 + ND],
            "emb_src": emb_src,
            "feature_src": ft,
            "W": W,
            "W2": W2,
        })
    res = run_bass_kernel_spmd(nc, in_maps, list(range(N_CORES)),
                               trace=_trace)
    out = np.concatenate([res.results[i]["out"] for i in range(N_CORES)], axis=0)
    if _trace:
        return out, res
    return out


# revision 12
# speedup vs baseline: 3.6549x; 1.0004x over previous
"""Trainium2 Bass kernel for nn_AttentionLayer (GAT-style masked attention).

Computes, for full inputs:
    h1 = emb_src @ W                      [8000, 128]
    g  = emb_dest @ (W @ W2)              [10000, 128]
    e  = g @ h1.T                         [10000, 8000]
    s  = lrelu(e, 0.2) * (1/sqrt(128))    masked to -inf where bias <= 0
    att = softmax(s, axis=1)
    out = att @ ft                        [10000, 1]   (ft = nan-cleaned feature_src)

Sharding: N_dest split across 8 NeuronCores (1250 rows each); emb_src /
feature_src / W / W2 replicated. No collectives. Softmax is computed
unnormalized (numer/denom) — no max-subtraction needed since |s| <= ~10.

Per-core device pipeline (per 128-row dest tile x 1000-col src slice):
    PE:     e_psum = gT.T @ h1T                       (bf16 x bf16 -> f32 PSUM)
    GPSIMD: amask  = (bias <= 0) * -1e30              (from streamed bias tile)
    DVE:    ms     = e_psum + amask
    ACT:    t      = Lrelu(SCALE * ms)  [alpha=0.2]
    ACT:    u      = Exp(t)             [accum_out -> denom partial]
    DVE:    ttr u * ft_bcast            [accum_out -> numer partial]
    out = numer / denom
"""
import os
import sys

sys.path.insert(0, "/opt/trn_rl_repo")

import numpy as np

_CACHE = {}

N_DEST, N_SRC, IN_DIM, HID = 10000, 8000, 256, 128
N_CORES = 8
ND = N_DEST // N_CORES            # 1250 dest rows per core
SCALE = float(1.0 / np.sqrt(np.float32(HID)))

# dest tiles per core: 9 x 128 + 98
DEST_TILES = [(i * 128, min(128, ND - i * 128)) for i in range((ND + 127) // 128)]
SRC_CHUNK = 500                   # matmul N (<= 512 = one PSUM bank of f32)
N_SRC_CHUNKS = N_SRC // SRC_CHUNK # 16
SS_W = 2 * SRC_CHUNK              # 1000-col superslice for ACT/DVE ops
N_SS = N_SRC // SS_W              # 8


def _build_nc():
    import concourse.bass as bass
    import concourse.tile as tile
    from concourse import bacc, mybir
    from concourse.masks import make_identity
    from contextlib import ExitStack

    F32 = mybir.dt.float32
    BF16 = mybir.dt.bfloat16
    FP16 = mybir.dt.float16
    AF = mybir.ActivationFunctionType
    OP = mybir.AluOpType

    nc = bacc.Bacc("TRN2", target_bir_lowering=False, debug=False,
                   num_devices=N_CORES)

    bias_t = nc.declare_dram_parameter("bias", [ND, N_SRC], F32, isOutput=False)
    dest_t = nc.declare_dram_parameter("emb_dest", [ND, IN_DIM], F32, isOutput=False)
    src_t = nc.declare_dram_parameter("emb_src", [N_SRC, IN_DIM], F32, isOutput=False)
    ft_t = nc.declare_dram_parameter("feature_src", [N_SRC, 1], F32, isOutput=False)
    w_t = nc.declare_dram_parameter("W", [IN_DIM, HID], F32, isOutput=False)
    w2_t = nc.declare_dram_parameter("W2", [HID, HID], F32, isOutput=False)
    out_t = nc.declare_dram_parameter("out", [ND, 1], F32, isOutput=True)

    with tile.TileContext(nc) as tc, ExitStack() as ctx:
        persist = ctx.enter_context(tc.tile_pool(name="persist", bufs=1))

        ident = persist.tile([128, 128], F32)
        make_identity(nc, ident)

        # ftbc: feature row broadcast across 128 partitions, then bf16
        ft_row = ft_t[:, :].rearrange("s one -> one s")   # [1, 8000] view
        ftbc = persist.tile([128, N_SRC], F32)
        nc.sync.dma_start(out=ftbc, in_=ft_row.to_broadcast([128, N_SRC]))
        ftbc_bf = persist.tile([128, N_SRC], BF16)
        nc.vector.tensor_copy(out=ftbc_bf, in_=ftbc)

        gt_tiles = []
        h1t_tiles = []

        with tc.tile_pool(name="pre_sb", bufs=3) as pre, \
             tc.tile_pool(name="pre_ps", bufs=2, space="PSUM") as pps:

            # ---- W chunks ([K=in_dim sub, M=hid]) natural + bf16
            w_sb = pre.tile([128, 2, HID], F32, tag="w_sb")
            for c in range(2):
                nc.sync.dma_start(out=w_sb[:, c, :], in_=w_t[128 * c:128 * (c + 1), :])
            w_bf = persist.tile([128, 2, HID], BF16)
            nc.vector.tensor_copy(out=w_bf, in_=w_sb)
            w2_sb = pre.tile([128, HID], F32, tag="w2_sb")
            nc.sync.dma_start(out=w2_sb, in_=w2_t[:, :])

            # ---- Wc = W @ W2, stored as lhsT chunks [K=in_dim sub, M=hid] bf16
            wc_bf = persist.tile([128, 2, HID], BF16)
            for c in range(2):
                ps_tr = pps.tile([128, 128], F32, tag="ps_a")
                nc.tensor.transpose(ps_tr, w_sb[:, c, :], ident)    # [hid, in_sub]
                wTc = pre.tile([128, 128], F32, tag="wTc")
                nc.scalar.copy(out=wTc, in_=ps_tr)
                ps_mm = pps.tile([128, HID], F32, tag="ps_b")
                nc.tensor.matmul(ps_mm, wTc, w2_sb, start=True, stop=True)
                nc.scalar.copy(out=wc_bf[:, c, :], in_=ps_mm)

            # ---- emb_dest -> destT (bf16, [in_sub, 2, dest]) -> gT tiles
            destT = pre.tile([128, 2, ND], BF16, tag="destT")
            for (r0, rn) in DEST_TILES:
                ed = pre.tile([128, IN_DIM], F32, tag="ed")
                nc.sync.dma_start(out=ed[:rn, :], in_=dest_t[r0:r0 + rn, :])
                for c in range(2):
                    ps_tr = pps.tile([128, 128], F32, tag="ps_a")
                    nc.tensor.transpose(ps_tr[:, :rn], ed[:rn, 128 * c:128 * (c + 1)],
                                        ident[:rn, :rn])
                    if c == 0:
                        nc.scalar.copy(out=destT[:, c, r0:r0 + rn], in_=ps_tr[:, :rn])
                    else:
                        nc.vector.tensor_copy(out=destT[:, c, r0:r0 + rn], in_=ps_tr[:, :rn])
            for ti, (r0, rn) in enumerate(DEST_TILES):
                ps_g = pps.tile([128, 128], F32, tag="ps_b")
                for c in range(2):
                    nc.tensor.matmul(ps_g[:, :rn], wc_bf[:, c, :],
                                     destT[:, c, r0:r0 + rn],
                                     start=(c == 0), stop=(c == 1))
                gt = persist.tile([128, 128], BF16, tag=f"gt{ti}")
                nc.scalar.copy(out=gt[:, :rn], in_=ps_g[:, :rn])
                gt_tiles.append(gt)

            # ---- emb_src -> srcT chunks -> h1T chunk tiles
            for j in range(N_SRC_CHUNKS):
                srcT = pre.tile([128, 2, SRC_CHUNK], BF16, tag="srcT")
                for k in range(4):                       # 4 x 125 src rows
                    s0 = j * SRC_CHUNK + k * 125
                    es = pre.tile([125, IN_DIM], F32, tag="es")
                    nc.sync.dma_start(out=es, in_=src_t[s0:s0 + 125, :])
                    for c in range(2):
                        ps_tr = pps.tile([128, 128], F32, tag="ps_a")
                        nc.tensor.transpose(ps_tr[:, :125],
                                            es[:, 128 * c:128 * (c + 1)],
                                            ident[:125, :125])
                        if (k + c) % 2 == 0:
                            nc.scalar.copy(out=srcT[:, c, 125 * k:125 * (k + 1)],
                                           in_=ps_tr[:, :125])
                        else:
                            nc.vector.tensor_copy(out=srcT[:, c, 125 * k:125 * (k + 1)],
                                                  in_=ps_tr[:, :125])
                ps_h = pps.tile([128, SRC_CHUNK], F32, tag="ps_b")
                for c in range(2):
                    nc.tensor.matmul(ps_h, w_bf[:, c, :], srcT[:, c, :],
                                     start=(c == 0), stop=(c == 1))
                h1t = persist.tile([128, SRC_CHUNK], BF16, tag=f"h1t{j}")
                nc.vector.tensor_copy(out=h1t, in_=ps_h)
                h1t_tiles.append(h1t)

        # ================= main loop =================
        with tc.tile_pool(name="mn_bias", bufs=4) as pbias, \
             tc.tile_pool(name="mn_mask", bufs=4) as pmask, \
             tc.tile_pool(name="mn_ms", bufs=3) as pms, \
             tc.tile_pool(name="mn_u", bufs=3) as pu, \
             tc.tile_pool(name="mn_small", bufs=2) as psm, \
             tc.tile_pool(name="mn_ps", bufs=3, space="PSUM") as mps:

            for ti, (r0, rn) in enumerate(DEST_TILES):
                gt = gt_tiles[ti]
                dpart = psm.tile([128, N_SS // 2], F32, tag="dpart")
                npart = psm.tile([128, N_SS // 2], F32, tag="npart")

                for p in range(N_SS // 2):
                    c0 = p * 2 * SS_W
                    btile = pbias.tile([128, 2 * SS_W], F32, tag="btile")
                    nc.sync.dma_start(out=btile[:rn, :],
                                      in_=bias_t[r0:r0 + rn, c0:c0 + 2 * SS_W])
                    amask = pmask.tile([128, 2 * SS_W], F32, tag="amask")
                    nc.vector.tensor_scalar(
                        out=amask[:rn, :], in0=btile[:rn, :],
                        scalar1=0.0, scalar2=-60000.0,
                        op0=OP.is_le, op1=OP.mult)

                    ms = pms.tile([128, 4, SRC_CHUNK], FP16, tag="ms")
                    for h in range(2):
                        ps_e = mps.tile([128, 2, 512], F32, tag="ps_e")
                        for q in range(2):
                            nc.tensor.matmul(
                                ps_e[:rn, q, 0:SRC_CHUNK],
                                gt[:, :rn], h1t_tiles[4 * p + 2 * h + q],
                                start=True, stop=True)
                        nc.vector.tensor_add(
                            ms[:rn, 2 * h:2 * h + 2, :],
                            ps_e[:rn, :, 0:SRC_CHUNK],
                            amask[:rn, h * SS_W:(h + 1) * SS_W].rearrange(
                                "p (b c) -> p b c", b=2))

                    msf = ms[:rn].rearrange("p b c -> p (b c)")
                    t0 = pms.tile([128, 2 * SS_W], FP16, tag="t0")
                    nc.vector.tensor_scalar_mul(t0[:rn, :], msf, 0.2)
                    t = pms.tile([128, 2 * SS_W], FP16, tag="t")
                    nc.vector.tensor_max(t[:rn, :], msf, t0[:rn, :])
                    u = pu.tile([128, 2 * SS_W], BF16, tag="u")
                    nc.scalar.activation(out=u[:rn, :], in_=t[:rn, :],
                                         func=AF.Exp, scale=SCALE,
                                         accum_out=dpart[:rn, p:p + 1])
                    prod = pu.tile([128, 2 * SS_W], BF16, tag="prod")
                    nc.vector.tensor_mul(prod[:rn, :], u[:rn, :],
                                         ftbc_bf[:rn, c0:c0 + 2 * SS_W])
                    scrap = pu.tile([128, 2 * SS_W], BF16, tag="scrap")
                    nc.scalar.activation(out=scrap[:rn, :], in_=prod[:rn, :],
                                         func=AF.Copy,
                                         accum_out=npart[:rn, p:p + 1])

                den = psm.tile([128, 1], F32, tag="den")
                nc.vector.tensor_reduce(den[:rn, :], dpart[:rn, :],
                                        axis=mybir.AxisListType.X, op=OP.add)
                num = psm.tile([128, 1], F32, tag="num")
                nc.vector.tensor_reduce(num[:rn, :], npart[:rn, :],
                                        axis=mybir.AxisListType.X, op=OP.add)
                rden = psm.tile([128, 1], F32, tag="rden")
                nc.vector.reciprocal(out=rden[:rn, :], in_=den[:rn, :])
                o = psm.tile([128, 1], F32, tag="o")
                nc.vector.tensor_mul(o[:rn, :], num[:rn, :], rden[:rn, :])
                nc.sync.dma_start(out=out_t[r0:r0 + rn, :], in_=o[:rn, :])

    nc.compile()
    return nc


def _get_nc():
    if "nc" not in _CACHE:
        _CACHE["nc"] = _build_nc()
    return _CACHE["nc"]


def kernel(bias, emb_dest, emb_src, feature_src, W, W2, _trace=False):
    from concourse.bass_utils import run_bass_kernel_spmd

    bias = np.ascontiguousarray(bias, dtype=np.float32)
    emb_dest = np.ascontiguousarray(emb_dest, dtype=np.float32)
    emb_src = np.ascontiguousarray(emb_src, dtype=np.float32)
    ft = np.ascontiguousarray(feature_src, dtype=np.float32)
    W = np.ascontiguousarray(W, dtype=np.float32)
    W2 = np.ascontiguousarray(W2, dtype=np.float32)

    nan_ind = np.isnan(ft.reshape(-1))
    if nan_ind.any():
        # NaN source features: zero the feature and mask out the column
        # (matches reference semantics). Never hit for randn inputs.
        ft = np.where(np.isnan(ft), 0.0, ft)
        bias = np.where(nan_ind.reshape(1, -1), -1.0, bias)

    nc = _get_nc()
    in_maps = []
    for i in range(N_CORES):
        r0 = i * ND
        in_maps.append({
            "bias": bias[r0:r0 + ND],
            "emb_dest": emb_dest[r0:r0 + ND],
            "emb_src": emb_src,
            "feature_src": ft,
            "W": W,
            "W2": W2,
        })
    res = run_bass_kernel_spmd(nc, in_maps, list(range(N_CORES)),
                               trace=_trace)
    out = np.concatenate([res.results[i]["out"] for i in range(N_CORES)], axis=0)
    if _trace:
        return out, res
    return out
